# revision 1
# baseline (speedup 1.0000x reference)
"""Trainium2 Bass kernel for nn_ExpertsChooseParallelBlock (MoNE expert-choice block).

Sharding: one batch element per NeuronCore (B=8 over 8 cores, no collectives).

Algorithmic collapse: w1/w2 are shared across experts; experts differ only by
(a) which tokens they pick (expert-choice top-cap over softmax router probs) and
(b) a nested channel-prefix mask m_e in {96,192,384,768}.  With cap = N/2 and the
scatter back to tokens being a sum, the whole dispatch/compute/combine pipeline
is equivalent to dense matmuls with per-token segment coefficients:

    sel[e,t] = probs[t,e] >= p512(e)     (p512 = 512-th largest of probs[:,e])
    c_j[t]   = sum_{e>=j} sel[e,t]        j=0..3  segments [0,96),[96,192),[192,384),[384,768)
    g_j[t]   = sum_{e>=j} sel[e,t]*probs[t,e]
    h   = w1 @ ((LN(x)*gamma+beta) * c)  + (sum_e sel)*b1      [5376 x N]
    y2  = [gelu(h_mlp); attn(h_qkv)]                            [3840 x N]
    O   = w2 @ y2 + b2                                          [1536 x N]
    out = x + g * (O[:768] + O[768:])

All heavy matmuls in bf16 with fp32 PSUM accumulation; router/softmax/threshold/
coefficients in fp32 so the selected token sets match the fp32 reference exactly.
"""

import numpy as np
import ml_dtypes

import concourse.bass as bass
from concourse import bacc
import concourse.tile as tile
import concourse.mybir as mybir
from concourse.masks import make_identity
from concourse.bass_utils import run_bass_kernel_spmd

F32 = mybir.dt.float32
F32R = mybir.dt.float32r
BF16 = mybir.dt.bfloat16
AF = mybir.ActivationFunctionType
ALU = mybir.AluOpType
AXX = mybir.AxisListType.X

DIM = 768
NE = 4
NH = 12
HD = 64
MLP = 3072
FC1 = 5376
FC2_IN = 3840
FC2_OUT = 1536
N = 1024
B = 8
LN_EPS = 1e-6
SEG = [0, 96, 192, 384, 768]
P = 128
KT1 = 6      # fc1 contraction tiles (768/128)
KT2 = 30     # fc2 contraction tiles (3840/128)
MT2 = 12     # fc2 out row tiles (1536/128)
NCH = 8      # token chunks (1024/128)

BF = ml_dtypes.bfloat16


PHASE_MARKS = []


def _emit(nc, tc, T, has_b1, has_b2, has_beta, has_gamma, loop_r=None):
    import contextlib

    def _mark(label):
        PHASE_MARKS.append((label, nc.next_id()))

    ctx = contextlib.ExitStack()
    singles = ctx.enter_context(tc.tile_pool(name="singles", bufs=1))
    small = ctx.enter_context(tc.tile_pool(name="small", bufs=2))
    wpool = ctx.enter_context(tc.tile_pool(name="wpool", bufs=2))
    w2pool = ctx.enter_context(tc.tile_pool(name="w2pool", bufs=2))
    qkpool = ctx.enter_context(tc.tile_pool(name="qkpool", bufs=2))
    espool = ctx.enter_context(tc.tile_pool(name="espool", bufs=2))
    opool = ctx.enter_context(tc.tile_pool(name="opool", bufs=1))
    ps = ctx.enter_context(tc.tile_pool(name="ps", bufs=4, space="PSUM"))

    # Wide tiles spanning TWO adjacent psum banks.  Halves t[:, h, :] are
    # bank-aligned matmul targets; full-width [P, 1024] reads are legal for
    # ACT/DVE/Pool, halving instruction counts.  Three pools so that fc1
    # m-tiles (a), long-lived accumulators (b) and the rotating stream (c)
    # never wait on each other's slots.
    def psum2(name):
        return ps.tile([P, 2, 512], F32, tag="bank2", name=name)

    def wide(t):
        return t[:].rearrange("p a b -> p (a b)")

    # Pre-size every pool tag up-front: the SBUF heap must not grow for an
    # earlier pool after a later pool has started allocating.
    small.tile([P, 512], BF16, tag="tmp512", name="presize1")
    small.tile([1, 2, 512], BF16, tag="tmp512", name="presize2")
    small.tile([P, N], BF16, tag="tmp512", name="presize3")
    if has_beta:
        small.tile([P, N], F32, tag="tmp512", name="presize4")
    small.tile([P, NCH], F32, tag="mx", name="presize5")
    small.tile([P, NCH], F32, tag="sm", name="presize6")
    small.tile([P, NCH, NE], F32, tag="sel", name="presize7")
    small.tile([P, NCH, NE], F32, tag="gate", name="presize8")
    small.tile([1, 7, NE], F32, tag="mlt", name="presize17")
    small.tile([1, NE], F32, tag="adv", name="presize18")
    wpool.tile([P, KT1 * P], BF16, tag="w1", name="presize9")
    w2pool.tile([P, KT2 * P], BF16, tag="w2a", name="presize10")
    w2pool.tile([P, KT2 * P], BF16, tag="w2b", name="presize11")
    qkpool.tile([P, N], BF16, tag="qt", name="presize12")
    qkpool.tile([P, N], BF16, tag="kt", name="presize13")
    espool.tile([P, 2, 512], BF16, tag="es", name="presize14")
    opool.tile([P, 2, 512], BF16, tag="gbs", name="presize15")
    opool.tile([P, 2, 512], BF16, tag="stage", name="presize16")

    if loop_r is not None:
        # Hardware loop re-executing the whole body loop_r times (used only by
        # timed_run to measure marginal per-iteration device time).
        ctx.enter_context(tc.For_i(0, loop_r))

    # ------------- resident inputs -------------
    xt = singles.tile([P, KT1, N], F32)            # x^T: [p, ct, t] = x[t, ct*128+p]
    for kt in range(KT1):
        nc.sync.dma_start(xt[:, kt, :], T["xT"][:, kt, :])
    wrts = singles.tile([P, KT1, NE], F32)
    nc.sync.dma_start(wrts[:], T["wrt"][:])
    e6s = singles.tile([NE, KT1, P], F32R)
    nc.sync.dma_start(e6s[:], T["e6"][:])
    ones_r = singles.tile([1, P], F32R)
    nc.sync.dma_start(ones_r[:], T["onesr"][:])
    gcs = singles.tile([P, KT1], F32)
    nc.sync.dma_start(gcs[:], T["gcol"][:])
    bcs = singles.tile([P, KT1], F32)
    nc.sync.dma_start(bcs[:], T["bcol"][:])
    b1s = singles.tile([P, 36], F32)
    nc.sync.dma_start(b1s[:], T["b1c"][:])
    b2s = singles.tile([P, MT2], F32)
    nc.sync.dma_start(b2s[:], T["b2c"][:])
    ident = singles.tile([P, P], F32)
    make_identity(nc, ident[:])
    eps_t = singles.tile([1, 1], F32)
    nc.vector.memset(eps_t[:], LN_EPS)

    ypsb = singles.tile([P, KT1, N], BF16)         # y'^T (fc1 rhs / V lhsT)
    w1vsb = singles.tile([P, KT1, DIM], BF16)
    nc.sync.dma_start(w1vsb[:], T["w1vp"][:])
    vaug = singles.tile([P, NCH, NH, HD + 1], BF16)  # V with ones column
    y2sb = singles.tile([P, KT2, N], BF16)         # fc2 rhs

    murow = singles.tile([1, N], F32)
    rstdrow = singles.tile([1, N], F32)
    ones4 = singles.tile([1, NE], F32)
    nc.vector.memset(ones4[:], 1.0)
    v = singles.tile([1, N], F32)
    probs = singles.tile([P, NCH, NE], F32)
    pcont = singles.tile([P, NE, NCH], F32)  # contiguous per-expert copies
    tlorow = singles.tile([1, NE], F32)
    ones_fr = singles.tile([1, P], F32)
    nc.vector.memset(ones_fr[:], 1.0)
    dconsts = singles.tile([1, 8, 7, 1], F32)
    nc.sync.dma_start(dconsts[:], T["dconst"][:])
    ones_f32 = singles.tile([P, 1], F32)
    nc.vector.memset(ones_f32[:], 1.0)
    cg = singles.tile([P, NCH, 8], F32)  # slots 0..3 c_j, 4..7 g_j
    cT = singles.tile([NE, NCH, P], F32R)
    gT = singles.tile([NE, NCH, P], F32R)
    crs = singles.tile([NE, NCH * P], F32R)      # c_j * rstd rows
    if has_b1:
        sbcs = singles.tile([P, N], F32)
        b1vr = singles.tile([1, DIM], F32)
        b1vbc = singles.tile([P, DIM], F32)

    _mark("ln_stats")
    # ------------- LN stats (mu, rstd rows): bf16 ones-matmuls -------------
    ones_b = singles.tile([P, 1], BF16)
    nc.vector.memset(ones_b[:], 1.0)

    mu_ps = psum2("mu")
    sq_ps = psum2("sq")
    for kt in range(KT1):
        for h in range(2):
            xbf = small.tile([P, 512], BF16, tag="tmp512")
            nc.scalar.activation(xbf[:], xt[:, kt, h * 512:(h + 1) * 512], AF.Copy)
            nc.tensor.matmul(mu_ps[0:1, h, :], ones_b[:], xbf[:],
                             start=(kt == 0), stop=(kt == KT1 - 1), skip_group_check=True)
            xsq = small.tile([P, 512], BF16, tag="tmp512")
            nc.scalar.activation(xsq[:], xt[:, kt, h * 512:(h + 1) * 512], AF.Square)
            nc.tensor.matmul(sq_ps[0:1, h, :], ones_b[:], xsq[:],
                             start=(kt == 0), stop=(kt == KT1 - 1), skip_group_check=True)
    nc.scalar.mul(murow[:], mu_ps[0:1, :, :].rearrange("p a b -> p (a b)"), 1.0 / DIM)
    # var = sumsq/768 - mu^2 ; rstd = 1/sqrt(var + eps)   (wide row ops)
    nc.vector.tensor_mul(v[:], murow[:], murow[:])
    nc.vector.scalar_tensor_tensor(v[:], sq_ps[0:1, :, :].rearrange("p a b -> p (a b)"),
                                   1.0 / DIM, v[:], op0=ALU.mult, op1=ALU.subtract)
    nc.scalar.activation(v[:], v[:], AF.Sqrt, bias=eps_t[:])
    nc.vector.reciprocal(v[:], v[:])
    nc.vector.tensor_copy(rstdrow[:], v[:])
    # mu broadcast [128, N] held in a wide PSUM tile (K=1 ones matmul);
    # gpsimd partition_broadcast costs ~2-5us on HW, the matmul ~1us.
    mub_ps = psum2("mub")
    for h in range(2):
        nc.tensor.matmul(mub_ps[:, h, :], ones_fr[:], murow[0:1, h * 512:(h + 1) * 512],
                         start=True, stop=True)
    mubs = singles.tile([P, N], F32)
    nc.vector.tensor_copy(mubs[:], wide(mub_ps))

    _mark("router")
    # ------------- router: logits -> probs (fp32, N-layout) -------------
    for c in range(NCH):
        lp = psum2("lg")
        for kt in range(KT1):
            nc.tensor.matmul(lp[:, 0, 0:NE], xt[:, kt, c * P:(c + 1) * P], wrts[:, kt, :],
                             start=(kt == 0), stop=(kt == KT1 - 1))
        nc.vector.tensor_copy(probs[:, c, :], lp[:, 0, 0:NE])
    mx = small.tile([P, NCH], F32, tag="mx")
    nc.vector.reduce_max(mx[:], probs[:], axis=AXX)
    nc.vector.tensor_sub(probs[:], probs[:], mx[:, :, None].to_broadcast((P, NCH, NE)))
    nc.scalar.activation(probs[:], probs[:], AF.Exp)
    sm = small.tile([P, NCH], F32, tag="sm")
    nc.vector.reduce_sum(sm[:], probs[:], axis=AXX)
    nc.vector.reciprocal(sm[:], sm[:])
    nc.vector.tensor_mul(probs[:], probs[:], sm[:, :, None].to_broadcast((P, NCH, NE)))

    _mark("threshold")
    # ------------- per-expert threshold = 512-th largest --------------------
    # gpsimd kth_largest costs ~370us/call on HW (Q7 software heap with
    # k=510 ~ n).  Instead: vectorized binary search on the threshold value,
    # all 4 experts at once.  Invariant: count(p >= lo) >= 512 > count(p >= hi).
    # 8 iterations x 3 bits = 24 bits of threshold resolution; the measured
    # minimum gap between the 512th/513th order statistics is 2.8e-6 (~2^-18.5),
    # so sel = (p >= lo) selects exactly the top-512 set with 40x margin.
    for e in range(NE):
        nc.vector.tensor_copy(pcont[:, e, :], probs[:, :, e])
    nc.vector.memset(tlorow[:], 0.0)
    NMID = 7
    for it in range(8):
        # m_j = lo + j*8^-(it+1), j=1..7, all experts at once (one row op)
        mrow = small.tile([1, NMID, NE], F32, tag="sm")
        nc.vector.tensor_add(mrow[:],
                             dconsts[0:1, it, :, :].to_broadcast((1, NMID, NE)),
                             tlorow[0:1, None, :].to_broadcast((1, NMID, NE)))
        mbc = psum2("mbc")
        nc.tensor.matmul(mbc[:, 0, 0:NMID * NE], ones_fr[:],
                         mrow[:].rearrange("p a e -> p (a e)"),
                         start=True, stop=True)
        ge3 = small.tile([P, NMID, NE, NCH], F32, tag="tmpN")
        nc.vector.tensor_tensor(
            ge3[:], pcont[:, None, :, :].to_broadcast((P, NMID, NE, NCH)),
            mbc[:, 0, 0:NMID * NE].rearrange("p (a e) -> p a e", e=NE)[:, :, :, None]
               .to_broadcast((P, NMID, NE, NCH)),
            ALU.is_ge)
        rsum = small.tile([P, NMID, NE], F32, tag="mx")
        nc.vector.reduce_sum(rsum[:], ge3[:], axis=AXX)
        nc.tensor.matmul(mbc[0:1, 1, 0:NMID * NE], ones_f32[:],
                         rsum[:].rearrange("p a e -> p (a e)"),
                         start=True, stop=True)
        mge3 = small.tile([1, NMID, NE], F32, tag="mlt")
        nc.vector.tensor_scalar(mge3[:], mbc[0:1, 1, 0:NMID * NE]
                                .rearrange("p (a e) -> p a e", e=NE),
                                512.0, None, op0=ALU.is_ge)
        adv = small.tile([1, NE], F32, tag="adv")
        nc.vector.reduce_sum(adv[:], mge3[:].rearrange("p a e -> p e a"), axis=AXX)
        nc.vector.scalar_tensor_tensor(tlorow[:], adv[:], 8.0 ** (-(it + 1)),
                                       tlorow[:], op0=ALU.mult, op1=ALU.add)
    # broadcast the final threshold row to all partitions
    tbc_ps = psum2("tbc")
    nc.tensor.matmul(tbc_ps[:, 0, 0:NE], ones_fr[:], tlorow[:],
                     start=True, stop=True)

    _mark("coeffs")
    # ------------- coefficients c_j, g_j (fp32) -------------
    sel = small.tile([P, NCH, NE], F32, tag="sel")
    nc.vector.tensor_tensor(sel[:], probs[:], tbc_ps[:, 0, 0:NE][:, None, :].to_broadcast((P, NCH, NE)),
                            ALU.is_ge)
    gate = small.tile([P, NCH, NE], F32, tag="gate")
    nc.vector.tensor_mul(gate[:], sel[:], probs[:])
    nc.vector.tensor_copy(cg[:, :, 3], sel[:, :, 3])
    nc.vector.tensor_copy(cg[:, :, 7], gate[:, :, 3])
    for j in (2, 1, 0):
        nc.vector.tensor_add(cg[:, :, j], cg[:, :, j + 1], sel[:, :, j])
        nc.vector.tensor_add(cg[:, :, 4 + j], cg[:, :, 4 + j + 1], gate[:, :, j])
    # transpose -> rows at partition base 0: cT[j, c*128+p], gT[j, c*128+p]
    for c in range(NCH):
        tp = psum2("cgt")
        nc.tensor.transpose(tp[0:NE, 0, 0:P], cg[:, c, 0:NE], ident[:])
        nc.tensor.transpose(tp[0:NE, 1, 0:P], cg[:, c, NE:2 * NE], ident[:])
        nc.vector.tensor_copy(cT[:, c, :], tp[0:NE, 0, 0:P])
        nc.vector.tensor_copy(gT[:, c, :], tp[0:NE, 1, 0:P])
    cTf = cT[:].rearrange("s c p -> s (c p)")
    grs = gT[:].rearrange("s c p -> s (c p)")
    crs_ps = psum2("crsbc")
    for h in range(2):
        nc.tensor.matmul(crs_ps[0:NE, h, :], ones4[:], rstdrow[0:1, h * 512:(h + 1) * 512],
                         start=True, stop=True)
    nc.vector.tensor_mul(crs[:], crs_ps[0:NE, :, :].rearrange("p a b -> p (a b)"),
                         cTf[:])

    _mark("yprime")
    # ------------- y' = (x - mu) * gamma * (c * rstd) (+ beta * c) -------------
    for ct in range(KT1):
        cb = psum2("crsb")
        for h in range(2):
            sl = slice(h * 512, (h + 1) * 512)
            nc.tensor.matmul(cb[:, h, :], e6s[:, ct, :], crs[:, sl],
                             start=True, stop=True)
        t0 = small.tile([P, N], BF16, tag="tmp512")
        nc.vector.tensor_sub(t0[:], xt[:, ct, :], mubs[:])
        if has_gamma:
            nc.vector.scalar_tensor_tensor(ypsb[:, ct, :], t0[:], gcs[:, ct:ct + 1],
                                           wide(cb), op0=ALU.mult, op1=ALU.mult)
        else:
            nc.vector.tensor_mul(ypsb[:, ct, :], t0[:], wide(cb))
        if has_beta:
            cbp = psum2("cbp")
            for h in range(2):
                sl = slice(h * 512, (h + 1) * 512)
                nc.tensor.matmul(cbp[:, h, :], e6s[:, ct, :], cTf[0:NE, sl],
                                 start=True, stop=True)
            bterm = small.tile([P, N], F32, tag="tmp512")
            nc.vector.scalar_tensor_tensor(bterm[:], wide(cbp), bcs[:, ct:ct + 1],
                                           ypsb[:, ct, :], op0=ALU.mult, op1=ALU.add)
            nc.vector.tensor_copy(ypsb[:, ct, :], bterm[:])

    # S broadcast for bias terms (S = c_0 = number of experts per token)
    if has_b1:
        for h in range(2):
            sb_ps = psum2("sbc")
            nc.tensor.matmul(sb_ps[:, 0, :], ones_r[:], cTf[0:1, h * 512:(h + 1) * 512],
                             start=True, stop=True)
            nc.vector.tensor_copy(sbcs[:, h * 512:(h + 1) * 512], sb_ps[:, 0, :])
        nc.sync.dma_start(b1vr[:], T["b1vr"][:])
        nc.gpsimd.partition_broadcast(b1vbc[:], b1vr[:])

    def fc1_mtile(m, dest_cb):
        """Compute h^T m-tile (rows m*128..) into a wide psum; dest_cb(pm)."""
        wm = wpool.tile([P, KT1 * P], BF16, tag="w1")
        nc.sync.dma_start(wm[:], T["w1p"][m])
        pm = psum2("pm")
        for kt in range(KT1):
            for h in range(2):
                nc.tensor.matmul(pm[:, h, :], wm[:, kt * P:(kt + 1) * P],
                                 ypsb[:, kt, h * 512:(h + 1) * 512],
                                 start=(kt == 0), stop=(kt == KT1 - 1),
                                 skip_group_check=True)
        if has_b1:
            nc.vector.scalar_tensor_tensor(wide(pm), sbcs[:], b1s[:, m:m + 1],
                                           wide(pm), op0=ALU.mult, op1=ALU.add)
        dest_cb(pm)

    def qk_tiles(i):
        qs = qkpool.tile([P, N], BF16, tag="qt", name=f"qs{i}")
        ks = qkpool.tile([P, N], BF16, tag="kt", name=f"ks{i}")
        fc1_mtile(24 + i, lambda pm: nc.vector.tensor_copy(qs[:], wide(pm)))
        fc1_mtile(30 + i, lambda pm: nc.vector.tensor_copy(ks[:], wide(pm)))
        return qs, ks


    _mark("fc1_v")
    # ------------- fc1: V part (N-layout, out [t, d]) -------------
    nc.vector.memset(vaug[:, :, :, HD], 1.0)
    nxt = None
    for mv in range(NCH):
        if mv == 4:
            nxt = qk_tiles(0)   # overlap first head-pair fc1 with V tail
        pv = psum2("pv")
        for kt in range(KT1):
            for h, width in ((0, 512), (1, 256)):
                nc.tensor.matmul(pv[:, h, 0:width], ypsb[:, kt, mv * P:(mv + 1) * P],
                                 w1vsb[:, kt, h * 512:h * 512 + width],
                                 start=(kt == 0), stop=(kt == KT1 - 1),
                                 skip_group_check=True)
        if has_b1:
            nc.vector.scalar_tensor_tensor(
                wide(pv)[:, 0:DIM], b1vbc[:], cg[:, mv, 0:1],
                wide(pv)[:, 0:DIM], op0=ALU.mult, op1=ALU.add)
        nc.vector.tensor_copy(
            vaug[:, mv, :, 0:HD],
            wide(pv)[:, 0:DIM].rearrange("p (nh d) -> p nh d", d=HD))

    _mark("attn")
    for i in range(6):  # head pairs (software-pipelined: next pair's q/k fc1
        # matmuls are emitted between this pair's two q-halves so they fill
        # the exp->AV drain at the pair boundary)
        qs, ks = nxt
        for qh in range(2):
            qsl = slice(qh * 512, (qh + 1) * 512)
            pav = psum2("pav")

            def do_av(pend_):
                kk_, e_ = pend_
                nc.tensor.matmul(pav[0:65, 0, :], vaug[:, kk_, 2 * i, :], e_[:, 0, :],
                                 start=(kk_ == 0), stop=(kk_ == NCH - 1),
                                 skip_group_check=True)
                nc.tensor.matmul(pav[0:65, 1, :], vaug[:, kk_, 2 * i + 1, :], e_[:, 1, :],
                                 start=(kk_ == 0), stop=(kk_ == NCH - 1),
                                 skip_group_check=True)

            pend = None
            for kk in range(NCH):
                sAB = psum2("sAB")
                nc.tensor.matmul(sAB[:, 0, :], ks[0:64, kk * P:(kk + 1) * P],
                                 qs[0:64, qsl], start=True, stop=True)
                nc.tensor.matmul(sAB[:, 1, :], ks[64:128, kk * P:(kk + 1) * P],
                                 qs[64:128, qsl], start=True, stop=True,
                                 tile_position=(64, 0))
                if pend is not None:
                    do_av(pend)
                es = espool.tile([P, 2, 512], BF16, tag="es")
                nc.scalar.activation(wide(es), wide(sAB), AF.Exp, scale=0.125)
                pend = (kk, es)
            do_av(pend)
            rrf = small.tile([1, 2, 512], BF16, tag="tmp512")
            with nc.allow_low_precision(reason="softmax denom reciprocal in bf16"):
                nc.vector.reciprocal(wide(rrf), pav[64:65, :, :].rearrange("p a b -> p (a b)"))
            dbs = opool.tile([P, 2, 512], BF16, tag="gbs")
            nc.gpsimd.partition_broadcast(wide(dbs), wide(rrf))
            nc.vector.tensor_mul(y2sb[0:64, 24 + i, qsl], pav[0:64, 0, :], dbs[0:64, 0, :])
            nc.vector.tensor_mul(y2sb[64:128, 24 + i, qsl], pav[0:64, 1, :], dbs[0:64, 1, :])
            if qh == 0 and i < 5:
                nxt = qk_tiles(i + 1)

    _mark("fc1_mlp")
    # ------------- fc1 MLP part -> gelu -> y2 -------------
    for m in range(24):
        def mlp_cb(pm, m=m):
            nc.scalar.activation(y2sb[:, m, :], wide(pm), AF.Gelu)
        fc1_mtile(m, mlp_cb)

    _mark("fc2")
    # ------------- fc2 + combine, m-pairs (j, j+6) -------------
    # O[:768] and O[768:] are summed, so rows j and j+6 accumulate into ONE
    # psum half (60 matmuls); both token-halves live in one wide tile.  gb
    # goes PSUM->SBUF on the Pool engine (DVE ops read at most one PSUM
    # operand), and the whole epilogue runs as wide [P, 1024] ops.
    for j in range(6):
        wa = w2pool.tile([P, KT2 * P], BF16, tag="w2a")
        wb = w2pool.tile([P, KT2 * P], BF16, tag="w2b")
        nc.sync.dma_start(wa[:], T["w2p"][j])
        nc.sync.dma_start(wb[:], T["w2p"][j + 6])
        oacc = psum2("oacc")
        for wsrc, first, last in ((wa, True, False), (wb, False, True)):
            for kt in range(KT2):
                for h in range(2):
                    nc.tensor.matmul(oacc[:, h, :], wsrc[:, kt * P:(kt + 1) * P],
                                     y2sb[:, kt, h * 512:(h + 1) * 512],
                                     start=(first and kt == 0),
                                     stop=(last and kt == KT2 - 1),
                                     skip_group_check=True)
        gbp = psum2("gb")
        for h in range(2):
            sl = slice(h * 512, (h + 1) * 512)
            nc.tensor.matmul(gbp[:, h, :], e6s[:, j, :], grs[:, sl],
                             start=True, stop=True)
        gbs = opool.tile([P, 2, 512], BF16, tag="gbs")
        with nc.allow_low_precision(reason="gate coeffs in bf16"):
            nc.vector.tensor_copy(wide(gbs), wide(gbp))
        stage = opool.tile([P, 2, 512], BF16, tag="stage")
        with nc.allow_low_precision(reason="bf16 stage; residual added in fp32 next"):
            if has_b2:
                nc.vector.tensor_scalar(wide(stage), wide(oacc), b2s[:, j:j + 1],
                                        b2s[:, j + 6:j + 7], op0=ALU.add, op1=ALU.add)
                nc.vector.tensor_mul(wide(stage), wide(stage), wide(gbs))
            else:
                nc.vector.tensor_mul(wide(stage), wide(oacc), wide(gbs))
        stage_o = small.tile([P, N], BF16, tag="tmp512")
        nc.vector.tensor_add(stage_o[:], wide(stage), xt[:, j, :])
        nc.sync.dma_start(T["outT"][:, j, :], stage_o[:])

    ctx.close()


_built = {}


def _build(flags, loop_r=None):
    key = (flags, loop_r)
    if key in _built:
        return _built[key]
    has_b1, has_b2, has_beta, has_gamma = flags
    nc = bacc.Bacc("TRN2", target_bir_lowering=False, debug=False)
    T = {}
    T["xT"] = nc.dram_tensor("xT", [P, KT1, N], F32, kind="ExternalInput")
    T["w1p"] = nc.dram_tensor("w1p", [36, P, KT1 * P], BF16, kind="ExternalInput")
    T["w1vp"] = nc.dram_tensor("w1vp", [P, KT1, DIM], BF16, kind="ExternalInput")
    T["w2p"] = nc.dram_tensor("w2p", [MT2, P, KT2 * P], BF16, kind="ExternalInput")
    T["wrt"] = nc.dram_tensor("wrt", [P, KT1, NE], F32, kind="ExternalInput")
    T["e6"] = nc.dram_tensor("e6", [NE, KT1, P], F32R, kind="ExternalInput")
    T["onesr"] = nc.dram_tensor("onesr", [1, P], F32R, kind="ExternalInput")
    T["onesf"] = nc.dram_tensor("onesf", [P, 1], F32, kind="ExternalInput")
    T["gcol"] = nc.dram_tensor("gcol", [P, KT1], F32, kind="ExternalInput")
    T["bcol"] = nc.dram_tensor("bcol", [P, KT1], F32, kind="ExternalInput")
    T["b1c"] = nc.dram_tensor("b1c", [P, 36], F32, kind="ExternalInput")
    T["b2c"] = nc.dram_tensor("b2c", [P, MT2], F32, kind="ExternalInput")
    T["dconst"] = nc.dram_tensor("dconst", [1, 8, 7, 1], F32,
                                 kind="ExternalInput")
    T["b1vr"] = nc.dram_tensor("b1vr", [1, DIM], F32, kind="ExternalInput")
    T["outT"] = nc.dram_tensor("outT", [P, KT1, N], BF16, kind="ExternalOutput")
    with tile.TileContext(nc) as tc:
        _emit(nc, tc, T, has_b1, has_b2, has_beta, has_gamma, loop_r=loop_r)
    nc.compile()
    _built[key] = nc
    return nc


def _seg_idx():
    s = np.zeros(DIM, dtype=np.int64)
    for j in range(NE):
        s[SEG[j]:SEG[j + 1]] = j
    return s


def _pack_inputs(x, w_router, gamma1, beta1, w1, b1, w2, b2):
    x = np.asarray(x, dtype=np.float32)
    w_router = np.asarray(w_router, dtype=np.float32)
    gamma1 = np.asarray(gamma1, dtype=np.float32)
    beta1 = np.asarray(beta1, dtype=np.float32)
    w1 = np.asarray(w1, dtype=np.float32)
    b1 = np.asarray(b1, dtype=np.float32)
    w2 = np.asarray(w2, dtype=np.float32)
    b2 = np.asarray(b2, dtype=np.float32)
    w1p = np.ascontiguousarray(
        w1[:4608].reshape(36, P, KT1, P).transpose(0, 3, 2, 1).reshape(36, P, KT1 * P)
    ).astype(BF)
    w1vp = np.ascontiguousarray(
        w1[4608:].reshape(DIM, KT1, P).transpose(2, 1, 0)).astype(BF)
    w2p = np.ascontiguousarray(
        w2.reshape(MT2, P, KT2, P).transpose(0, 3, 2, 1).reshape(MT2, P, KT2 * P)
    ).astype(BF)
    wrt = np.ascontiguousarray(w_router.T.reshape(KT1, P, NE).transpose(1, 0, 2))
    sj = _seg_idx()
    e6 = np.zeros((NE, KT1, P), dtype=np.float32)
    for ct in range(KT1):
        for p in range(P):
            e6[sj[ct * P + p], ct, p] = 1.0
    onesr = np.ones((1, P), dtype=np.float32)
    onesf = np.ones((P, 1), dtype=np.float32)
    gcol = np.ascontiguousarray(gamma1.reshape(KT1, P).T)
    bcol = np.ascontiguousarray(beta1.reshape(KT1, P).T)
    b1c = np.ascontiguousarray(b1[:4608].reshape(36, P).T)
    b2c = np.ascontiguousarray(b2.reshape(MT2, P).T)
    b1vr = np.ascontiguousarray(b1[4608:].reshape(1, DIM))

    shared = dict(w1p=w1p, w1vp=w1vp, w2p=w2p, wrt=wrt, e6=e6, onesr=onesr,
                  onesf=onesf, gcol=gcol, bcol=bcol, b1c=b1c, b2c=b2c, b1vr=b1vr)
    in_maps = []
    for b in range(B):
        xT = np.ascontiguousarray(
            x[b].T.reshape(KT1, P, N).transpose(1, 0, 2))
        m = dict(shared)
        m["xT"] = xT
        in_maps.append(m)

    return in_maps


# ---------------------------------------------------------------------------
# Execution: persistent jitted shard_map executable + device-resident inputs.
#
# run_bass_kernel_spmd builds a fresh jit closure per call (cache miss every
# time -> retrace + re-serialize BIR + PJRT compile-cache lookup) and ships
# every input over the axon tunnel (~35 MB/s) on every call.  We instead build
# the jitted callable once per flag-set and keep inputs resident on device,
# re-uploading a tensor only when its content hash changes.
# ---------------------------------------------------------------------------

_EXEC_CACHE = {}
_DEV_STATE = {"dev": {}, "digests": {}}   # device-resident inputs, shared


def _get_exec(flags, loop_r=None):
    key = (flags, loop_r)
    if key in _EXEC_CACHE:
        return _EXEC_CACHE[key]
    import jax
    from jax.sharding import Mesh, PartitionSpec, NamedSharding
    from jax.experimental.shard_map import shard_map
    from concourse import bass2jax
    import concourse.mybir as mb

    bass2jax.install_neuronx_cc_hook()
    nc = _build(flags, loop_r=loop_r)
    assert nc.dbg_addr is None

    partition_name = nc.partition_id_tensor.name if nc.partition_id_tensor else None
    in_names, out_names, out_avals = [], [], []
    for alloc in nc.m.functions[0].allocations:
        if not isinstance(alloc, mb.MemoryLocationSet):
            continue
        name = alloc.memorylocations[0].name
        if alloc.kind == "ExternalInput":
            if name != partition_name:
                in_names.append(name)
        elif alloc.kind == "ExternalOutput":
            out_names.append(name)
            out_avals.append(jax.core.ShapedArray(
                tuple(alloc.tensor_shape), mybir.dt.np(alloc.dtype)))
    n_params = len(in_names)
    all_in = in_names + out_names
    if partition_name is not None:
        all_in = all_in + [partition_name]
    donate = tuple(range(n_params, n_params + len(out_names)))

    def _body(*args):
        operands = list(args)
        if partition_name is not None:
            operands.append(bass2jax.partition_id_tensor())
        outs = bass2jax._bass_exec_p.bind(
            *operands,
            out_avals=tuple(out_avals),
            in_names=tuple(all_in),
            out_names=tuple(out_names),
            lowering_input_output_aliases=(),
            sim_require_finite=True,
            sim_require_nnan=True,
            nc=nc,
        )
        return tuple(outs)

    devices = jax.devices()[:B]
    mesh = Mesh(np.asarray(devices), ("core",))
    spec = NamedSharding(mesh, PartitionSpec("core"))
    n_outs = len(out_names)
    sharded = jax.jit(
        shard_map(_body, mesh=mesh,
                  in_specs=(PartitionSpec("core"),) * (n_params + n_outs),
                  out_specs=(PartitionSpec("core"),) * n_outs,
                  check_rep=False),
        donate_argnums=donate, keep_unused=True)

    def _zeros():
        return tuple(jax.numpy.zeros((B * a.shape[0], *a.shape[1:]), a.dtype)
                     for a in out_avals)
    zeros_fn = jax.jit(_zeros, out_shardings=(spec,) * n_outs)

    ex = dict(nc=nc, sharded=sharded, zeros_fn=zeros_fn, spec=spec,
              in_names=in_names, out_names=out_names, out_avals=out_avals,
              dev=_DEV_STATE["dev"], digests=_DEV_STATE["digests"],
              recycle=None)
    _EXEC_CACHE[key] = ex
    return ex


def _digest(*arrays):
    # sha256 in 4 threads (hashlib releases the GIL on large updates)
    import hashlib
    from concurrent.futures import ThreadPoolExecutor
    bufs = [np.ascontiguousarray(a).view(np.uint8).data for a in arrays]
    with ThreadPoolExecutor(4) as tp:
        digs = list(tp.map(
            lambda b: hashlib.sha256(b).digest(), bufs))
    return b"".join(digs)


def _pack_weights(w_router, gamma1, beta1, w1, b1, w2, b2):
    w1p = np.ascontiguousarray(
        w1[:4608].reshape(36, P, KT1, P).transpose(0, 3, 2, 1).reshape(36, P, KT1 * P)
    ).astype(BF)
    w1vp = np.ascontiguousarray(
        w1[4608:].reshape(DIM, KT1, P).transpose(2, 1, 0)).astype(BF)
    w2p = np.ascontiguousarray(
        w2.reshape(MT2, P, KT2, P).transpose(0, 3, 2, 1).reshape(MT2, P, KT2 * P)
    ).astype(BF)
    wrt = np.ascontiguousarray(w_router.T.reshape(KT1, P, NE).transpose(1, 0, 2))
    sj = _seg_idx()
    e6 = np.zeros((NE, KT1, P), dtype=np.float32)
    for ct in range(KT1):
        for p in range(P):
            e6[sj[ct * P + p], ct, p] = 1.0
    onesr = np.ones((1, P), dtype=np.float32)
    onesf = np.ones((P, 1), dtype=np.float32)
    gcol = np.ascontiguousarray(gamma1.reshape(KT1, P).T)
    bcol = np.ascontiguousarray(beta1.reshape(KT1, P).T)
    b1c = np.ascontiguousarray(b1[:4608].reshape(36, P).T)
    b2c = np.ascontiguousarray(b2.reshape(MT2, P).T)
    b1vr = np.ascontiguousarray(b1[4608:].reshape(1, DIM))
    dconst = np.zeros((1, 8, 7, 1), dtype=np.float32)
    for i in range(8):
        for j in range(7):
            dconst[0, i, j, 0] = (j + 1) * 8.0 ** (-(i + 1))
    return dict(w1p=w1p, w1vp=w1vp, w2p=w2p, wrt=wrt, e6=e6, onesr=onesr,
                onesf=onesf, gcol=gcol, bcol=bcol, b1c=b1c, b2c=b2c, b1vr=b1vr,
                dconst=dconst)


def _pack_x(x):
    # x [B, N, DIM] -> per-core xT [P, KT1, N], concatenated on axis 0
    xt = np.ascontiguousarray(
        x.transpose(0, 2, 1).reshape(B, KT1, P, N).transpose(0, 2, 1, 3))
    return xt.reshape(B * P, KT1, N)


def _upload(ex, name, np_global):
    import jax
    ex["dev"][name] = jax.device_put(np_global, ex["spec"])


def _upload_all(ex, x, w_router, gamma1, beta1, w1, b1, w2, b2):
    wd = _digest(w_router, gamma1, beta1, w1, b1, w2, b2)
    if ex["digests"].get("w") != wd:
        shared = _pack_weights(w_router, gamma1, beta1, w1, b1, w2, b2)
        for name, arr in shared.items():
            rep = np.ascontiguousarray(
                np.broadcast_to(arr[None], (B,) + arr.shape)
            ).reshape(B * arr.shape[0], *arr.shape[1:])
            _upload(ex, name, rep)
        ex["digests"]["w"] = wd
    xd = _digest(x)
    if ex["digests"].get("x") != xd:
        _upload(ex, "xT", _pack_x(x))
        ex["digests"]["x"] = xd


def _dispatch(ex):
    """Launch the resident executable (async). The donated dummy operands for
    the output slots recycle the previous call's output arrays."""
    if ex["recycle"] is None:
        ex["recycle"] = list(ex["zeros_fn"]())
    args = [ex["dev"][n] for n in ex["in_names"]] + ex["recycle"]
    out_arrs = ex["sharded"](*args)
    ex["recycle"] = list(out_arrs)
    return out_arrs


def _flags_of(b1, b2, beta1, gamma1):
    return (bool(np.any(b1 != 0)), bool(np.any(b2 != 0)),
            bool(np.any(beta1 != 0)), bool(np.any(gamma1 != 1)))


def kernel(x, w_router, gamma1, beta1, w1, b1, w2, b2):
    x = np.asarray(x, dtype=np.float32)
    w_router = np.asarray(w_router, dtype=np.float32)
    gamma1 = np.asarray(gamma1, dtype=np.float32)
    beta1 = np.asarray(beta1, dtype=np.float32)
    w1 = np.asarray(w1, dtype=np.float32)
    b1 = np.asarray(b1, dtype=np.float32)
    w2 = np.asarray(w2, dtype=np.float32)
    b2 = np.asarray(b2, dtype=np.float32)
    ex = _get_exec(_flags_of(b1, b2, beta1, gamma1))
    _upload_all(ex, x, w_router, gamma1, beta1, w1, b1, w2, b2)
    out_arrs = _dispatch(ex)
    arr = np.asarray(out_arrs[0]).reshape(B, P, KT1, N)   # [b, p, ct, t], bf16
    return np.ascontiguousarray(
        arr.transpose(0, 3, 2, 1).astype(np.float32)).reshape(B, N, DIM)


LOOP_R = 65


def timed_run(inputs):
    """Honest per-invocation device execution time (ns), measured on hardware.

    The axon client has no NTFF profiling hook, so a single dispatch wall-time
    is dominated by the ~80 ms RPC round trip.  Instead we compile a second
    NEFF whose body is the identical kernel wrapped in a hardware For_i loop
    (LOOP_R iterations, same I/O), and report

        (min_wall(loop NEFF) - min_wall(single NEFF)) / (LOOP_R - 1)

    All dispatch/transfer overhead is identical between the two, so the delta
    is pure device execution time of (LOOP_R-1) kernel iterations."""
    import time
    ins = {k: np.asarray(v, np.float32) for k, v in inputs.items()}
    kernel(**ins)  # compile + upload + warm
    flags = _flags_of(ins["b1"], ins["b2"], ins["beta1"], ins["gamma1"])
    ex1 = _get_exec(flags)
    exL = _get_exec(flags, loop_r=LOOP_R)

    def min_wall(ex, n=12):
        best = 1e30
        for _ in range(n):
            t0 = time.time()
            out = _dispatch(ex)
            for o in out:
                o.block_until_ready()
            best = min(best, time.time() - t0)
        return best

    min_wall(ex1, n=2)  # warm both executables
    min_wall(exL, n=2)
    t1 = min_wall(ex1)
    tL = min_wall(exL)
    return int(max(tL - t1, 0.0) * 1e9 / (LOOP_R - 1))

